# revision 2
# baseline (speedup 1.0000x reference)
"""Trainium2 Bass kernel for BinderEnergyGuidance (retrieval_knn), v5.

Per batch b of 16:
  d[b,n,m]   = ||binder[b,n] - target[m]||           (N=1024, M=8192)
  attract[b] = mean of the k=204 smallest per-row min-distances
  repel[b]   = sum relu(3 - d)^2
  out[b]     = 10*attract[b] + 5*repel[b]

Data-parallel over batch: 2 batches/core.  PE computes d2 into PSUM
(fp32r, K packed at partition groups 0/32/64), 4 psum tiles of
[128,2048] per chunk (128 rows x 8192 cols).  Every chunk is HYBRID so
VectorE and ScalarE both get steady work at a tunable ratio:

  U-part (XV tiles, VectorE drains): q = min(d2,9) bf16 (accum
    per-tile row-min), then u = max(q,1e-4) span pass (accum row-sum,
    4x bf16); ScalarE sqrt(u) span (accum row-sum dc).
    repel part = 9*sz + sum(u) - 6*sum(dc), exact 0 per non-clash
    element (bf16 u=9.0 -> dc=3.0 exactly).
  Q-part (4-XV tiles, ScalarE drains): dcraw = sqrt(d2+EPS) bf16;
    VectorE dcc = min(dcraw,3) (accum row-min distance); ScalarE
    Square(dcc-3) span (accum row-sum = repel part, exact 0 per
    non-clash element).

Span activations of chunk c are deferred until after chunk c+1's
ScalarE psum drain, so PE is never blocked behind them.  Staging:
zero-fills on GPSIMD memset; y/x arithmetic split VectorE/ScalarE.
Top-k attract via rank selection against a DMA-broadcast row of
min-dists; final partition sum via DMA flatten + reduce.

Self-contained: hardcodes shapes binder[16,1024,3], target[8192,3].
"""

import numpy as np
from contextlib import ExitStack

import concourse.bass as bass
import concourse.bacc as bacc
import concourse.tile as tile
from concourse import mybir
from concourse.bass_utils import run_bass_kernel_spmd

F32 = mybir.dt.float32
BF16 = mybir.dt.bfloat16
F32R = mybir.dt.float32r
AF = mybir.ActivationFunctionType
OP = mybir.AluOpType
AX = mybir.AxisListType

B, N, MT = 16, 1024, 8192
NCORES = 8
BC = B // NCORES
TOPK = 204
ATTRACT_SCALE, REPEL_SCALE = 10.0, 5.0

P = 128
NCHUNK = N // P           # 8
MTILE = 2048              # PSUM tile (4 banks)
NMT = MT // MTILE         # 4
MMF = 512
KP = 67

EPS = 0.001
XV = 3                    # V-drained psum tiles per chunk (1..3)
UW = XV * MTILE           # U-part width
QW = MT - UW              # Q-part width

_prog_cache = {}


def build_program():
    nc = bacc.Bacc("TRN2", target_bir_lowering=False, debug=False,
                   num_devices=NCORES)
    bnd = nc.dram_tensor("bnd", [BC, 3, N], F32, kind="ExternalInput").ap()
    tgt = nc.dram_tensor("tgt", [3, MT], F32, kind="ExternalInput").ap()
    out = nc.dram_tensor("out", [BC, 1], F32, kind="ExternalOutput").ap()

    with tile.TileContext(nc) as tc, ExitStack() as ctx:
        consts = ctx.enter_context(tc.tile_pool(name="consts", bufs=1))
        work = ctx.enter_context(tc.tile_pool(name="work", bufs=1))
        tcp = ctx.enter_context(tc.tile_pool(name="tcp", bufs=2))
        psum = ctx.enter_context(tc.tile_pool(name="psum", bufs=2, space="PSUM"))
        dpool = ctx.enter_context(tc.tile_pool(name="dpool", bufs=1, space="DRAM"))

        rhs_subs = [consts.tile([KP, MTILE], F32R, name=f"rhs{k}")
                    for k in range(NMT)]
        lhsTs = [consts.tile([KP, N], F32R, name=f"lhsT_pad{b}")
                 for b in range(BC)]
        with tc.tile_pool(name="zscr", bufs=1) as zscr:
            for k in range(NMT):
                nc.gpsimd.memset(
                    rhs_subs[k][:, :].bitcast(mybir.dt.uint32), 0)
            for b in range(BC):
                nc.gpsimd.memset(
                    lhsTs[b][:, :].bitcast(mybir.dt.uint32), 0)
            for k in range(NMT):
                ksl = slice(k * MTILE, (k + 1) * MTILE)
                ysh = zscr.tile([3, MTILE], F32, name="ysh", tag="ysh")
                nc.sync.dma_start(out=ysh[:, :], in_=tgt[:, ksl])
                rs = rhs_subs[k]
                if k % 2 == 0:
                    nc.vector.tensor_scalar_mul(rs[0:3, :], ysh[:, :], -2.0)
                    nc.vector.tensor_mul(rs[64:67, :], ysh[:, :], ysh[:, :])
                    nc.vector.tensor_scalar(rs[32:35, :], ysh[:, :], 0.0, 1.0,
                                            OP.mult, OP.add)
                else:
                    nc.scalar.activation(rs[0:3, :], ysh[:, :], AF.Copy,
                                         bias=0.0, scale=-2.0)
                    nc.scalar.activation(rs[64:67, :], ysh[:, :], AF.Square)
                    nc.scalar.activation(rs[32:35, :], ysh[:, :], AF.Copy,
                                         bias=1.0, scale=0.0)
            for b in range(BC):
                xs = zscr.tile([3, N], F32, name=f"xs{b}")
                nc.sync.dma_start(out=xs[:, :], in_=bnd[b, :, :])
                lhsT_pad = lhsTs[b]
                nc.vector.tensor_copy(lhsT_pad[0:3, :], xs[:, :])
                nc.vector.tensor_mul(lhsT_pad[32:35, :], xs[:, :], xs[:, :])
                nc.vector.tensor_scalar(lhsT_pad[64:67, :], xs[:, :], 0.0, 1.0,
                                        OP.mult, OP.add)

        beps = consts.tile([P, 1], F32)
        nc.vector.memset(beps, EPS)
        bm3 = consts.tile([P, 1], F32)
        nc.vector.memset(bm3, -3.0)

        act_waste = work.tile([P, MT], BF16, name="act_waste")
        waste_r = work.tile([P, N], BF16, name="waste_r")

        pending = []

        for b in range(BC):
            lhsT = lhsTs[b]
            mdUB = work.tile([P, NCHUNK], F32, name=f"mdUB{b}")
            mdQB = work.tile([P, NCHUNK], F32, name=f"mdQB{b}")
            suB = work.tile([P, NCHUNK], F32, name=f"suB{b}")
            sdB = work.tile([P, NCHUNK], F32, name=f"sdB{b}")
            sqB = work.tile([P, NCHUNK], F32, name=f"sqB{b}")

            for c in range(NCHUNK):
                lc = lhsT[:, c * P:(c + 1) * P]
                span_u = tcp.tile([P, UW], BF16, name="span_u", tag="span_u")
                span_d = tcp.tile([P, QW], BF16, name="span_d", tag="span_d")
                us = tcp.tile([P, UW], BF16, name="us", tag="us")
                dcc = tcp.tile([P, QW], BF16, name="dcc", tag="dcc")
                mq = tcp.tile([P, XV], F32, name="mq", tag="mq")
                for k in range(NMT):
                    ps = psum.tile([P, MTILE], F32, name="ps", tag="ps")
                    for q in range(MTILE // MMF):
                        nc.tensor.matmul(
                            ps[:, q * MMF:(q + 1) * MMF], lc,
                            rhs_subs[k][:, q * MMF:(q + 1) * MMF],
                            start=True, stop=True)
                    if k < NMT - XV:
                        # act-drained Q tile
                        nc.scalar.activation(
                            span_d[:, k * MTILE:(k + 1) * MTILE], ps,
                            AF.Sqrt, bias=beps, scale=1.0)
                        if pending:
                            pending.pop(0)()
                    else:
                        kk = k - (NMT - XV)
                        nc.vector.tensor_scalar(
                            span_u[:, kk * MTILE:(kk + 1) * MTILE], ps,
                            9.0, 3.4e38, OP.min, OP.min,
                            accum_out=mq[:, kk:kk + 1])
                nc.vector.tensor_scalar(us, span_u, 1e-4, 0.0,
                                        OP.max, OP.add,
                                        accum_out=suB[:, c:c + 1])
                nc.vector.tensor_scalar(dcc, span_d, 3.0, 3.4e38,
                                        OP.min, OP.min,
                                        accum_out=mdQB[:, c:c + 1])
                nc.vector.tensor_reduce(mdUB[:, c:c + 1], mq, AX.X, OP.min)

                def mk(us_=us, dcc_=dcc, cs=c):
                    def go_sqrt():
                        nc.scalar.activation(act_waste[:, 0:UW], us_,
                                             AF.Sqrt,
                                             accum_out=sdB[:, cs:cs + 1])

                    def go_sq():
                        nc.scalar.activation(act_waste[:, UW:MT], dcc_,
                                             AF.Square, bias=bm3, scale=1.0,
                                             accum_out=sqB[:, cs:cs + 1])
                    return [go_sqrt, go_sq]
                pending.extend(mk())

            while pending:
                pending.pop(0)()

            # ---- per-batch epilogue ----
            vB = work.tile([P, NCHUNK], F32, name=f"vB{b}")
            mdc = work.tile([P, NCHUNK], F32, name=f"mdc{b}")
            nc.vector.tensor_scalar(mdc, mdUB, 1e-6, None, OP.max)
            vBu = work.tile([P, NCHUNK], F32, name=f"vBu{b}")
            nc.scalar.activation(vBu, mdc, AF.Sqrt)
            nc.vector.tensor_tensor(vB, vBu, mdQB, OP.min)

            vBb = work.tile([P, NCHUNK], BF16, name=f"vBb{b}")
            nc.vector.tensor_copy(vBb, vB)
            vfl = dpool.tile([1, N], BF16, name=f"vfl{b}")
            nc.sync.dma_start(
                out=vfl[0:1, :].rearrange("p (q c) -> p q c", q=P),
                in_=vBb[:, :])
            vrep = work.tile([P, N], BF16, name=f"vrep{b}")
            vfl_bcast = bass.AP(tensor=vfl.tensor, offset=vfl.offset,
                                ap=[[0, P], vfl.ap[-1]])
            nc.sync.dma_start(out=vrep[:, :], in_=vfl_bcast)

            rank8 = work.tile([P, NCHUNK], F32, name=f"rank8{b}")
            for c in range(NCHUNK):
                nc.vector.tensor_scalar(waste_r, vrep, vB[:, c:c + 1], 0.0,
                                        OP.is_lt, OP.add,
                                        accum_out=rank8[:, c:c + 1])
            sel8 = work.tile([P, NCHUNK], F32, name=f"sel8{b}")
            nc.vector.tensor_scalar(sel8, rank8, float(TOPK), None, OP.is_lt)
            prod8 = work.tile([P, NCHUNK], F32, name=f"prod8{b}")
            nc.vector.tensor_mul(prod8, sel8, vB)

            stack2 = work.tile([P, 2], F32, name=f"stack2{b}")
            nc.vector.tensor_reduce(stack2[:, 0:1], prod8, AX.X, OP.add)

            ru = work.tile([P, 1], F32, name=f"ru{b}")
            nc.vector.tensor_reduce(ru, suB, AX.X, OP.add)
            rd = work.tile([P, 1], F32, name=f"rd{b}")
            nc.vector.tensor_reduce(rd, sdB, AX.X, OP.add)
            rqq = work.tile([P, 1], F32, name=f"rqq{b}")
            nc.vector.tensor_reduce(rqq, sqB, AX.X, OP.add)
            t0 = work.tile([P, 1], F32, name=f"t0{b}")
            nc.vector.tensor_scalar(t0, rd, -6.0, None, OP.mult)
            t1 = work.tile([P, 1], F32, name=f"t1{b}")
            nc.vector.tensor_add(t1, t0, ru)
            t2 = work.tile([P, 1], F32, name=f"t2{b}")
            nc.vector.tensor_add(t2, t1, rqq)
            nc.vector.tensor_scalar(stack2[:, 1:2], t2,
                                    float(9.0 * UW * NCHUNK), None, OP.add)

            sfl = dpool.tile([1, 2 * P], F32, name=f"sfl{b}")
            nc.sync.dma_start(
                out=sfl[0:1, :].rearrange("p (q c) -> p q c", q=P),
                in_=stack2[:, :])
            ssb = work.tile([1, 2 * P], F32, name=f"ssb{b}")
            nc.sync.dma_start(out=ssb, in_=sfl)
            esum = work.tile([1, 2], F32, name=f"esum{b}")
            nc.vector.tensor_reduce(
                esum[0:1, :], ssb[0:1, :].rearrange("p (q c) -> p c q", c=2),
                AX.X, OP.add)
            en = work.tile([1, 2], F32, name=f"en{b}")
            nc.vector.tensor_scalar_mul(en[0:1, 0:1], esum[0:1, 0:1],
                                        ATTRACT_SCALE / TOPK)
            nc.vector.tensor_scalar_mul(en[0:1, 1:2], esum[0:1, 1:2],
                                        REPEL_SCALE)
            en2 = work.tile([1, 1], F32, name=f"en2{b}")
            nc.vector.tensor_add(en2, en[0:1, 0:1], en[0:1, 1:2])
            nc.sync.dma_start(out=out[b:b + 1, 0:1], in_=en2[0:1, 0:1])

    nc.compile()
    return nc


def _get_program():
    if "nc" not in _prog_cache:
        _prog_cache["nc"] = build_program()
    return _prog_cache["nc"]


def make_in_maps(binder_trans, target_coords):
    x = np.ascontiguousarray(
        np.asarray(binder_trans, dtype=np.float32).transpose(0, 2, 1))
    y = np.ascontiguousarray(np.asarray(target_coords, dtype=np.float32).T)
    return [{"bnd": np.ascontiguousarray(x[c * BC:(c + 1) * BC]), "tgt": y}
            for c in range(NCORES)]


def kernel(binder_trans, target_coords):
    nc = _get_program()
    in_maps = make_in_maps(binder_trans, target_coords)
    res = run_bass_kernel_spmd(nc, in_maps, list(range(NCORES)))
    outs = [np.asarray(res.results[c]["out"], dtype=np.float32).reshape(BC)
            for c in range(NCORES)]
    return np.concatenate(outs).astype(np.float32)


# revision 3
# speedup vs baseline: 1.0223x; 1.0223x over previous
"""Trainium2 Bass kernel for BinderEnergyGuidance (retrieval_knn), v5.

Per batch b of 16:
  d[b,n,m]   = ||binder[b,n] - target[m]||           (N=1024, M=8192)
  attract[b] = mean of the k=204 smallest per-row min-distances
  repel[b]   = sum relu(3 - d)^2
  out[b]     = 10*attract[b] + 5*repel[b]

Data-parallel over batch: 2 batches/core.  PE computes d2 into PSUM
(fp32r, K packed at partition groups 0/32/64), 4 psum tiles of
[128,2048] per chunk (128 rows x 8192 cols).  Every chunk is HYBRID so
VectorE and ScalarE both get steady work at a tunable ratio:

  U-part (XV tiles, VectorE drains): q = min(d2,9) bf16 (accum
    per-tile row-min), then u = max(q,1e-4) span pass (accum row-sum,
    4x bf16); ScalarE sqrt(u) span (accum row-sum dc).
    repel part = 9*sz + sum(u) - 6*sum(dc), exact 0 per non-clash
    element (bf16 u=9.0 -> dc=3.0 exactly).
  Q-part (4-XV tiles, ScalarE drains): dcraw = sqrt(d2+EPS) bf16;
    VectorE dcc = min(dcraw,3) (accum row-min distance); ScalarE
    Square(dcc-3) span (accum row-sum = repel part, exact 0 per
    non-clash element).

Span activations of chunk c are deferred until after chunk c+1's
ScalarE psum drain, so PE is never blocked behind them.  Staging:
zero-fills on GPSIMD memset; y/x arithmetic split VectorE/ScalarE.
Top-k attract via rank selection against a DMA-broadcast row of
min-dists; final partition sum via DMA flatten + reduce.

Self-contained: hardcodes shapes binder[16,1024,3], target[8192,3].
"""

import numpy as np
from contextlib import ExitStack

import concourse.bass as bass
import concourse.bacc as bacc
import concourse.tile as tile
from concourse import mybir
from concourse.bass_utils import run_bass_kernel_spmd

F32 = mybir.dt.float32
BF16 = mybir.dt.bfloat16
F32R = mybir.dt.float32r
AF = mybir.ActivationFunctionType
OP = mybir.AluOpType
AX = mybir.AxisListType

B, N, MT = 16, 1024, 8192
NCORES = 8
BC = B // NCORES
TOPK = 204
ATTRACT_SCALE, REPEL_SCALE = 10.0, 5.0

P = 128
NCHUNK = N // P           # 8
MTILE = 2048              # PSUM tile (4 banks)
NMT = MT // MTILE         # 4
MMF = 512
KP = 67

EPS = 0.001
PSUM_BUFS = 2
XV = 3                    # V-drained psum tiles per chunk (1..3)
UW = XV * MTILE           # U-part width
QW = MT - UW              # Q-part width

_prog_cache = {}


def build_program():
    nc = bacc.Bacc("TRN2", target_bir_lowering=False, debug=False,
                   num_devices=NCORES)
    bnd = nc.dram_tensor("bnd", [BC, 3, N], F32, kind="ExternalInput").ap()
    tgt = nc.dram_tensor("tgt", [3, MT], F32, kind="ExternalInput").ap()
    out = nc.dram_tensor("out", [BC, 1], F32, kind="ExternalOutput").ap()

    with tile.TileContext(nc) as tc, ExitStack() as ctx:
        consts = ctx.enter_context(tc.tile_pool(name="consts", bufs=1))
        work = ctx.enter_context(tc.tile_pool(name="work", bufs=1))
        tcp = ctx.enter_context(tc.tile_pool(name="tcp", bufs=2))
        psum = ctx.enter_context(tc.tile_pool(name="psum", bufs=PSUM_BUFS, space="PSUM"))
        dpool = ctx.enter_context(tc.tile_pool(name="dpool", bufs=1, space="DRAM"))

        rhs_subs = [consts.tile([KP, MTILE], F32R, name=f"rhs{k}")
                    for k in range(NMT)]
        lhsTs = [consts.tile([KP, N], F32R, name=f"lhsT_pad{b}")
                 for b in range(BC)]
        with tc.tile_pool(name="zscr", bufs=1) as zscr:
            for k in range(NMT):
                nc.gpsimd.memset(
                    rhs_subs[k][:, :].bitcast(mybir.dt.uint32), 0)
            for b in range(BC):
                nc.gpsimd.memset(
                    lhsTs[b][:, :].bitcast(mybir.dt.uint32), 0)
            def stage_lhs(b, eng):
                xs = zscr.tile([3, N], F32, name=f"xs{b}")
                nc.sync.dma_start(out=xs[:, :], in_=bnd[b, :, :])
                lhsT_pad = lhsTs[b]
                if eng == 'V':
                    nc.vector.tensor_copy(lhsT_pad[0:3, :], xs[:, :])
                    nc.vector.tensor_mul(lhsT_pad[32:35, :], xs[:, :],
                                         xs[:, :])
                    nc.vector.tensor_scalar(lhsT_pad[64:67, :], xs[:, :],
                                            0.0, 1.0, OP.mult, OP.add)
                else:
                    nc.scalar.activation(lhsT_pad[0:3, :], xs[:, :], AF.Copy)
                    nc.scalar.activation(lhsT_pad[32:35, :], xs[:, :],
                                         AF.Square)
                    nc.scalar.activation(lhsT_pad[64:67, :], xs[:, :],
                                         AF.Copy, bias=1.0, scale=0.0)

            def stage_rhs(k, eng):
                ksl = slice(k * MTILE, (k + 1) * MTILE)
                ysh = zscr.tile([3, MTILE], F32, name="ysh", tag="ysh")
                nc.sync.dma_start(out=ysh[:, :], in_=tgt[:, ksl])
                rs = rhs_subs[k]
                if eng == 'V':
                    nc.vector.tensor_scalar_mul(rs[0:3, :], ysh[:, :], -2.0)
                    nc.vector.tensor_mul(rs[64:67, :], ysh[:, :], ysh[:, :])
                    nc.vector.tensor_scalar(rs[32:35, :], ysh[:, :], 0.0, 1.0,
                                            OP.mult, OP.add)
                else:
                    nc.scalar.activation(rs[0:3, :], ysh[:, :], AF.Copy,
                                         bias=0.0, scale=-2.0)
                    nc.scalar.activation(rs[64:67, :], ysh[:, :], AF.Square)
                    nc.scalar.activation(rs[32:35, :], ysh[:, :], AF.Copy,
                                         bias=1.0, scale=0.0)

            stage_lhs(0, 'V')
            stage_rhs(0, 'act')
            stage_rhs(1, 'V')
            stage_rhs(2, 'act')
            stage_rhs(3, 'V')
            stage_lhs(1, 'act')

        ones128 = consts.tile([P, 1], F32)
        nc.vector.memset(ones128, 1.0)
        beps = consts.tile([P, 1], F32)
        nc.vector.memset(beps, EPS)
        bm3 = consts.tile([P, 1], F32)
        nc.vector.memset(bm3, -3.0)

        act_waste = work.tile([P, MT], BF16, name="act_waste")
        waste_r = work.tile([P, N], BF16, name="waste_r")

        pending = []

        for b in range(BC):
            lhsT = lhsTs[b]
            mdUB = work.tile([P, NCHUNK], F32, name=f"mdUB{b}")
            mdQB = work.tile([P, NCHUNK], F32, name=f"mdQB{b}")
            suB = work.tile([P, NCHUNK], F32, name=f"suB{b}")
            sdB = work.tile([P, NCHUNK], F32, name=f"sdB{b}")
            sqB = work.tile([P, NCHUNK], F32, name=f"sqB{b}")

            for c in range(NCHUNK):
                lc = lhsT[:, c * P:(c + 1) * P]
                span_u = tcp.tile([P, UW], BF16, name="span_u", tag="span_u")
                span_d = tcp.tile([P, QW], BF16, name="span_d", tag="span_d")
                us = tcp.tile([P, UW], BF16, name="us", tag="us")
                dcc = tcp.tile([P, QW], BF16, name="dcc", tag="dcc")
                mq = tcp.tile([P, XV], F32, name="mq", tag="mq")
                for k in range(NMT):
                    ps = psum.tile([P, MTILE], F32, name="ps", tag="ps")
                    for q in range(MTILE // MMF):
                        nc.tensor.matmul(
                            ps[:, q * MMF:(q + 1) * MMF], lc,
                            rhs_subs[k][:, q * MMF:(q + 1) * MMF],
                            start=True, stop=True)
                    if k < NMT - XV:
                        # act-drained Q tile
                        nc.scalar.activation(
                            span_d[:, k * MTILE:(k + 1) * MTILE], ps,
                            AF.Sqrt, bias=beps, scale=1.0)
                        for _ in range(min(2, len(pending))):
                            pending.pop(0)()
                    else:
                        kk = k - (NMT - XV)
                        nc.vector.tensor_scalar(
                            span_u[:, kk * MTILE:(kk + 1) * MTILE], ps,
                            9.0, 3.4e38, OP.min, OP.min,
                            accum_out=mq[:, kk:kk + 1])
                nc.vector.tensor_scalar(us, span_u, 1e-4, 0.0,
                                        OP.max, OP.add,
                                        accum_out=suB[:, c:c + 1])
                nc.vector.tensor_scalar(dcc, span_d, 3.0, 3.4e38,
                                        OP.min, OP.min,
                                        accum_out=mdQB[:, c:c + 1])
                nc.vector.tensor_reduce(mdUB[:, c:c + 1], mq, AX.X, OP.min)

                def mk(us_=us, dcc_=dcc, cs=c):
                    def go_sqrt():
                        nc.scalar.activation(act_waste[:, 0:UW], us_,
                                             AF.Sqrt,
                                             accum_out=sdB[:, cs:cs + 1])

                    def go_sq():
                        nc.scalar.activation(act_waste[:, UW:MT], dcc_,
                                             AF.Square, bias=bm3, scale=1.0,
                                             accum_out=sqB[:, cs:cs + 1])
                    return [go_sqrt, go_sq]
                pending.extend(mk())

            while pending:
                pending.pop(0)()

            # ---- per-batch epilogue ----
            vB = work.tile([P, NCHUNK], F32, name=f"vB{b}")
            mdc = work.tile([P, NCHUNK], F32, name=f"mdc{b}")
            nc.vector.tensor_scalar(mdc, mdUB, 1e-6, None, OP.max)
            vBu = work.tile([P, NCHUNK], F32, name=f"vBu{b}")
            nc.scalar.activation(vBu, mdc, AF.Sqrt)
            nc.vector.tensor_tensor(vB, vBu, mdQB, OP.min)

            vBb = work.tile([P, NCHUNK], BF16, name=f"vBb{b}")
            nc.vector.tensor_copy(vBb, vB)
            vfl = dpool.tile([1, N], BF16, name=f"vfl{b}")
            nc.sync.dma_start(
                out=vfl[0:1, :].rearrange("p (q c) -> p q c", q=P),
                in_=vBb[:, :])
            vrep = work.tile([P, N], BF16, name=f"vrep{b}")
            vfl_bcast = bass.AP(tensor=vfl.tensor, offset=vfl.offset,
                                ap=[[0, P], vfl.ap[-1]])
            nc.sync.dma_start(out=vrep[:, :], in_=vfl_bcast)

            rank8 = work.tile([P, NCHUNK], F32, name=f"rank8{b}")
            for c in range(NCHUNK):
                nc.vector.tensor_scalar(waste_r, vrep, vB[:, c:c + 1], 0.0,
                                        OP.is_lt, OP.add,
                                        accum_out=rank8[:, c:c + 1])
            sel8 = work.tile([P, NCHUNK], F32, name=f"sel8{b}")
            nc.vector.tensor_scalar(sel8, rank8, float(TOPK), None, OP.is_lt)
            prod8 = work.tile([P, NCHUNK], F32, name=f"prod8{b}")
            nc.vector.tensor_mul(prod8, sel8, vB)

            stack2 = work.tile([P, 2], F32, name=f"stack2{b}")
            nc.vector.tensor_reduce(stack2[:, 0:1], prod8, AX.X, OP.add)

            ru = work.tile([P, 1], F32, name=f"ru{b}")
            nc.vector.tensor_reduce(ru, suB, AX.X, OP.add)
            rd = work.tile([P, 1], F32, name=f"rd{b}")
            nc.vector.tensor_reduce(rd, sdB, AX.X, OP.add)
            rqq = work.tile([P, 1], F32, name=f"rqq{b}")
            nc.vector.tensor_reduce(rqq, sqB, AX.X, OP.add)
            t0 = work.tile([P, 1], F32, name=f"t0{b}")
            nc.vector.tensor_scalar(t0, rd, -6.0, None, OP.mult)
            t1 = work.tile([P, 1], F32, name=f"t1{b}")
            nc.vector.tensor_add(t1, t0, ru)
            t2 = work.tile([P, 1], F32, name=f"t2{b}")
            nc.vector.tensor_add(t2, t1, rqq)
            nc.vector.tensor_scalar(stack2[:, 1:2], t2,
                                    float(9.0 * UW * NCHUNK), None, OP.add)

            sfl = dpool.tile([1, 2 * P], F32, name=f"sfl{b}")
            nc.sync.dma_start(
                out=sfl[0:1, :].rearrange("p (q c) -> p q c", q=P),
                in_=stack2[:, :])
            ssb = work.tile([1, 2 * P], F32, name=f"ssb{b}")
            nc.sync.dma_start(out=ssb, in_=sfl)
            esum = work.tile([1, 2], F32, name=f"esum{b}")
            nc.vector.tensor_reduce(
                esum[0:1, :], ssb[0:1, :].rearrange("p (q c) -> p c q", c=2),
                AX.X, OP.add)
            en = work.tile([1, 2], F32, name=f"en{b}")
            nc.vector.tensor_scalar_mul(en[0:1, 0:1], esum[0:1, 0:1],
                                        ATTRACT_SCALE / TOPK)
            nc.vector.tensor_scalar_mul(en[0:1, 1:2], esum[0:1, 1:2],
                                        REPEL_SCALE)
            en2 = work.tile([1, 1], F32, name=f"en2{b}")
            nc.vector.tensor_add(en2, en[0:1, 0:1], en[0:1, 1:2])
            nc.sync.dma_start(out=out[b:b + 1, 0:1], in_=en2[0:1, 0:1])

    nc.compile()
    return nc


def _get_program():
    if "nc" not in _prog_cache:
        _prog_cache["nc"] = build_program()
    return _prog_cache["nc"]


def make_in_maps(binder_trans, target_coords):
    x = np.ascontiguousarray(
        np.asarray(binder_trans, dtype=np.float32).transpose(0, 2, 1))
    y = np.ascontiguousarray(np.asarray(target_coords, dtype=np.float32).T)
    return [{"bnd": np.ascontiguousarray(x[c * BC:(c + 1) * BC]), "tgt": y}
            for c in range(NCORES)]


def kernel(binder_trans, target_coords):
    nc = _get_program()
    in_maps = make_in_maps(binder_trans, target_coords)
    res = run_bass_kernel_spmd(nc, in_maps, list(range(NCORES)))
    outs = [np.asarray(res.results[c]["out"], dtype=np.float32).reshape(BC)
            for c in range(NCORES)]
    return np.concatenate(outs).astype(np.float32)


# revision 5
# speedup vs baseline: 1.0601x; 1.0369x over previous
"""Trainium2 Bass kernel for BinderEnergyGuidance (retrieval_knn), v5.

Per batch b of 16:
  d[b,n,m]   = ||binder[b,n] - target[m]||           (N=1024, M=8192)
  attract[b] = mean of the k=204 smallest per-row min-distances
  repel[b]   = sum relu(3 - d)^2
  out[b]     = 10*attract[b] + 5*repel[b]

Data-parallel over batch: 2 batches/core.  PE computes d2 into PSUM
(fp32r, K packed at partition groups 0/32/64), 4 psum tiles of
[128,2048] per chunk (128 rows x 8192 cols).  Every chunk is HYBRID so
VectorE and ScalarE both get steady work at a tunable ratio:

  U-part (XV tiles, VectorE drains): q = min(d2,9) bf16 (accum
    per-tile row-min), then u = max(q,1e-4) span pass (accum row-sum,
    4x bf16); ScalarE sqrt(u) span (accum row-sum dc).
    repel part = 9*sz + sum(u) - 6*sum(dc), exact 0 per non-clash
    element (bf16 u=9.0 -> dc=3.0 exactly).
  Q-part (4-XV tiles, ScalarE drains): dcraw = sqrt(d2+EPS) bf16;
    VectorE dcc = min(dcraw,3) (accum row-min distance); ScalarE
    Square(dcc-3) span (accum row-sum = repel part, exact 0 per
    non-clash element).

Span activations of chunk c are deferred until after chunk c+1's
ScalarE psum drain, so PE is never blocked behind them.  Staging:
zero-fills on GPSIMD memset; y/x arithmetic split VectorE/ScalarE.
Top-k attract via rank selection against a DMA-broadcast row of
min-dists; final partition sum via DMA flatten + reduce.

Self-contained: hardcodes shapes binder[16,1024,3], target[8192,3].
"""

import numpy as np
from contextlib import ExitStack

import concourse.bass as bass
import concourse.bacc as bacc
import concourse.tile as tile
from concourse import mybir
from concourse.bass_utils import run_bass_kernel_spmd

F32 = mybir.dt.float32
BF16 = mybir.dt.bfloat16
F32R = mybir.dt.float32r
AF = mybir.ActivationFunctionType
OP = mybir.AluOpType
AX = mybir.AxisListType

B, N, MT = 16, 1024, 8192
NCORES = 8
BC = B // NCORES
TOPK = 204
ATTRACT_SCALE, REPEL_SCALE = 10.0, 5.0

P = 128
NCHUNK = N // P           # 8
MTILE = 2048              # PSUM tile (4 banks)
NMT = MT // MTILE         # 4
MMF = 512
KP = 67

EPS = 0.001
PSUM_BUFS = 2
XV = 3                    # V-drained psum tiles per chunk (1..3)
UW = XV * MTILE           # U-part width
QW = MT - UW              # Q-part width

_prog_cache = {}


def build_program():
    nc = bacc.Bacc("TRN2", target_bir_lowering=False, debug=False,
                   num_devices=NCORES)
    bnd = nc.dram_tensor("bnd", [BC, 3, N], F32, kind="ExternalInput").ap()
    tgt = nc.dram_tensor("tgt", [3, MT], F32, kind="ExternalInput").ap()
    out = nc.dram_tensor("out", [BC, 1], F32, kind="ExternalOutput").ap()

    with tile.TileContext(nc) as tc, ExitStack() as ctx:
        consts = ctx.enter_context(tc.tile_pool(name="consts", bufs=1))
        work = ctx.enter_context(tc.tile_pool(name="work", bufs=1))
        tcp = ctx.enter_context(tc.tile_pool(name="tcp", bufs=2))
        psum = ctx.enter_context(tc.tile_pool(name="psum", bufs=PSUM_BUFS, space="PSUM"))
        dpool = ctx.enter_context(tc.tile_pool(name="dpool", bufs=1, space="DRAM"))

        rhs_subs = [consts.tile([KP, MTILE], F32R, name=f"rhs{k}")
                    for k in range(NMT)]
        lhsTs = [consts.tile([KP, N], F32R, name=f"lhsT_pad{b}")
                 for b in range(BC)]
        with tc.tile_pool(name="zscr", bufs=1) as zscr:
            nc.gpsimd.memset(lhsTs[0][:, :].bitcast(mybir.dt.uint32), 0)
            for k in range(NMT):
                nc.gpsimd.memset(
                    rhs_subs[k][:, :].bitcast(mybir.dt.uint32), 0)
            nc.gpsimd.memset(lhsTs[1][:, :].bitcast(mybir.dt.uint32), 0)
            def stage_lhs(b, eng):
                xs = zscr.tile([3, N], F32, name=f"xs{b}")
                nc.sync.dma_start(out=xs[:, :], in_=bnd[b, :, :])
                lhsT_pad = lhsTs[b]
                if eng == 'V':
                    nc.vector.tensor_copy(lhsT_pad[0:3, :], xs[:, :])
                    nc.vector.tensor_mul(lhsT_pad[32:35, :], xs[:, :],
                                         xs[:, :])
                    nc.vector.tensor_scalar(lhsT_pad[64:67, :], xs[:, :],
                                            0.0, 1.0, OP.mult, OP.add)
                else:
                    nc.scalar.activation(lhsT_pad[0:3, :], xs[:, :], AF.Copy)
                    nc.scalar.activation(lhsT_pad[32:35, :], xs[:, :],
                                         AF.Square)
                    nc.scalar.activation(lhsT_pad[64:67, :], xs[:, :],
                                         AF.Copy, bias=1.0, scale=0.0)

            def stage_rhs(k, eng):
                ksl = slice(k * MTILE, (k + 1) * MTILE)
                ysh = zscr.tile([3, MTILE], F32, name="ysh", tag="ysh")
                nc.sync.dma_start(out=ysh[:, :], in_=tgt[:, ksl])
                rs = rhs_subs[k]
                if eng == 'V':
                    nc.vector.tensor_scalar_mul(rs[0:3, :], ysh[:, :], -2.0)
                    nc.vector.tensor_mul(rs[64:67, :], ysh[:, :], ysh[:, :])
                    nc.vector.tensor_scalar(rs[32:35, :], ysh[:, :], 0.0, 1.0,
                                            OP.mult, OP.add)
                else:
                    nc.scalar.activation(rs[0:3, :], ysh[:, :], AF.Copy,
                                         bias=0.0, scale=-2.0)
                    nc.scalar.activation(rs[64:67, :], ysh[:, :], AF.Square)
                    nc.scalar.activation(rs[32:35, :], ysh[:, :], AF.Copy,
                                         bias=1.0, scale=0.0)

            stage_lhs(0, 'V')
            stage_rhs(0, 'act')
            stage_rhs(1, 'V')
            stage_rhs(2, 'act')
            stage_rhs(3, 'V')
            stage_lhs(1, 'act')

        ones128 = consts.tile([P, 1], F32)
        nc.vector.memset(ones128, 1.0)
        beps = consts.tile([P, 1], F32)
        nc.vector.memset(beps, EPS)
        bm3 = consts.tile([P, 1], F32)
        nc.vector.memset(bm3, -3.0)

        act_waste = work.tile([P, MT], BF16, name="act_waste")
        waste_r = work.tile([P, N], BF16, name="waste_r")

        pending = []

        for b in range(BC):
            lhsT = lhsTs[b]
            mdUB = work.tile([P, NCHUNK], F32, name=f"mdUB{b}")
            mdQB = work.tile([P, NCHUNK], F32, name=f"mdQB{b}")
            suB = work.tile([P, NCHUNK], F32, name=f"suB{b}")
            sdB = work.tile([P, NCHUNK], F32, name=f"sdB{b}")
            sqB = work.tile([P, NCHUNK], F32, name=f"sqB{b}")

            for c in range(NCHUNK):
                lc = lhsT[:, c * P:(c + 1) * P]
                span_u = tcp.tile([P, UW], BF16, name="span_u", tag="span_u")
                span_d = tcp.tile([P, QW], BF16, name="span_d", tag="span_d")
                us = tcp.tile([P, UW], BF16, name="us", tag="us")
                dcc = tcp.tile([P, QW], BF16, name="dcc", tag="dcc")
                mq = tcp.tile([P, XV], F32, name="mq", tag="mq")
                for k in range(NMT):
                    ps = psum.tile([P, MTILE], F32, name="ps", tag="ps")
                    for q in range(MTILE // MMF):
                        nc.tensor.matmul(
                            ps[:, q * MMF:(q + 1) * MMF], lc,
                            rhs_subs[k][:, q * MMF:(q + 1) * MMF],
                            start=True, stop=True)
                    if k < NMT - XV:
                        # act-drained Q tile
                        nc.scalar.activation(
                            span_d[:, k * MTILE:(k + 1) * MTILE], ps,
                            AF.Sqrt, bias=beps, scale=1.0)
                        for _ in range(min(2, len(pending))):
                            pending.pop(0)()
                    else:
                        kk = k - (NMT - XV)
                        nc.vector.tensor_scalar(
                            span_u[:, kk * MTILE:(kk + 1) * MTILE], ps,
                            9.0, 3.4e38, OP.min, OP.min,
                            accum_out=mq[:, kk:kk + 1])
                nc.vector.tensor_scalar(us, span_u, 1e-4, 0.0,
                                        OP.max, OP.add,
                                        accum_out=suB[:, c:c + 1])
                nc.vector.tensor_scalar(dcc, span_d, 3.0, 3.4e38,
                                        OP.min, OP.min,
                                        accum_out=mdQB[:, c:c + 1])
                nc.vector.tensor_reduce(mdUB[:, c:c + 1], mq, AX.X, OP.min)

                def mk(us_=us, dcc_=dcc, cs=c):
                    def go_sqrt():
                        nc.scalar.activation(act_waste[:, 0:UW], us_,
                                             AF.Sqrt,
                                             accum_out=sdB[:, cs:cs + 1])

                    def go_sq():
                        nc.scalar.activation(act_waste[:, UW:MT], dcc_,
                                             AF.Square, bias=bm3, scale=1.0,
                                             accum_out=sqB[:, cs:cs + 1])
                    return [go_sqrt, go_sq]
                pending.extend(mk())

            # ---- per-batch epilogue ----
            vB = work.tile([P, NCHUNK], F32, name=f"vB{b}")
            mdc = work.tile([P, NCHUNK], F32, name=f"mdc{b}")
            nc.vector.tensor_scalar(mdc, mdUB, 1e-6, None, OP.max)
            vBu = work.tile([P, NCHUNK], F32, name=f"vBu{b}")
            nc.scalar.activation(vBu, mdc, AF.Sqrt)
            nc.vector.tensor_tensor(vB, vBu, mdQB, OP.min)

            vBb = work.tile([P, NCHUNK], BF16, name=f"vBb{b}")
            nc.vector.tensor_copy(vBb, vB)
            vfl = dpool.tile([1, N], BF16, name=f"vfl{b}")
            nc.sync.dma_start(
                out=vfl[0:1, :].rearrange("p (q c) -> p q c", q=P),
                in_=vBb[:, :])
            vrep = work.tile([P, N], BF16, name=f"vrep{b}")
            vfl_bcast = bass.AP(tensor=vfl.tensor, offset=vfl.offset,
                                ap=[[0, P], vfl.ap[-1]])
            nc.sync.dma_start(out=vrep[:, :], in_=vfl_bcast)

            while pending:
                pending.pop(0)()
            rank8 = work.tile([P, NCHUNK], F32, name=f"rank8{b}")
            for c in range(NCHUNK):
                nc.vector.tensor_scalar(waste_r, vrep, vB[:, c:c + 1], 0.0,
                                        OP.is_lt, OP.add,
                                        accum_out=rank8[:, c:c + 1])
            sel8 = work.tile([P, NCHUNK], F32, name=f"sel8{b}")
            nc.vector.tensor_scalar(sel8, rank8, float(TOPK), None, OP.is_lt)
            prod8 = work.tile([P, NCHUNK], F32, name=f"prod8{b}")
            nc.vector.tensor_mul(prod8, sel8, vB)

            stack2 = work.tile([P, 2], F32, name=f"stack2{b}")
            nc.vector.tensor_reduce(stack2[:, 0:1], prod8, AX.X, OP.add)

            ru = work.tile([P, 1], F32, name=f"ru{b}")
            nc.vector.tensor_reduce(ru, suB, AX.X, OP.add)
            rd = work.tile([P, 1], F32, name=f"rd{b}")
            nc.vector.tensor_reduce(rd, sdB, AX.X, OP.add)
            rqq = work.tile([P, 1], F32, name=f"rqq{b}")
            nc.vector.tensor_reduce(rqq, sqB, AX.X, OP.add)
            t0 = work.tile([P, 1], F32, name=f"t0{b}")
            nc.vector.tensor_scalar(t0, rd, -6.0, None, OP.mult)
            t1 = work.tile([P, 1], F32, name=f"t1{b}")
            nc.vector.tensor_add(t1, t0, ru)
            t2 = work.tile([P, 1], F32, name=f"t2{b}")
            nc.vector.tensor_add(t2, t1, rqq)
            nc.vector.tensor_scalar(stack2[:, 1:2], t2,
                                    float(9.0 * UW * NCHUNK), None, OP.add)

            if b == BC - 1:
                # no matmuls follow: psum rotation can't stall
                fin = psum.tile([P, MTILE], F32, name="ps", tag="ps")
                nc.tensor.matmul(fin[0:1, 0:2], ones128, stack2,
                                 start=True, stop=True)
                esrc = fin[0:1, 0:2]
            else:
                sfl = dpool.tile([1, 2 * P], F32, name=f"sfl{b}")
                nc.sync.dma_start(
                    out=sfl[0:1, :].rearrange("p (q c) -> p q c", q=P),
                    in_=stack2[:, :])
                ssb = work.tile([1, 2 * P], F32, name=f"ssb{b}")
                nc.sync.dma_start(out=ssb, in_=sfl)
                esum = work.tile([1, 2], F32, name=f"esum{b}")
                nc.vector.tensor_reduce(
                    esum[0:1, :],
                    ssb[0:1, :].rearrange("p (q c) -> p c q", c=2),
                    AX.X, OP.add)
                esrc = esum[0:1, :]
            en = work.tile([1, 2], F32, name=f"en{b}")
            nc.vector.tensor_scalar_mul(en[0:1, 0:1], esrc[0:1, 0:1],
                                        ATTRACT_SCALE / TOPK)
            nc.vector.tensor_scalar_mul(en[0:1, 1:2], esrc[0:1, 1:2],
                                        REPEL_SCALE)
            en2 = work.tile([1, 1], F32, name=f"en2{b}")
            nc.vector.tensor_add(en2, en[0:1, 0:1], en[0:1, 1:2])
            nc.sync.dma_start(out=out[b:b + 1, 0:1], in_=en2[0:1, 0:1])

    nc.compile()
    return nc


def _get_program():
    if "nc" not in _prog_cache:
        _prog_cache["nc"] = build_program()
    return _prog_cache["nc"]


def make_in_maps(binder_trans, target_coords):
    x = np.ascontiguousarray(
        np.asarray(binder_trans, dtype=np.float32).transpose(0, 2, 1))
    y = np.ascontiguousarray(np.asarray(target_coords, dtype=np.float32).T)
    return [{"bnd": np.ascontiguousarray(x[c * BC:(c + 1) * BC]), "tgt": y}
            for c in range(NCORES)]


def kernel(binder_trans, target_coords):
    nc = _get_program()
    in_maps = make_in_maps(binder_trans, target_coords)
    res = run_bass_kernel_spmd(nc, in_maps, list(range(NCORES)))
    outs = [np.asarray(res.results[c]["out"], dtype=np.float32).reshape(BC)
            for c in range(NCORES)]
    return np.concatenate(outs).astype(np.float32)


# revision 7
# speedup vs baseline: 1.0698x; 1.0092x over previous
"""Trainium2 Bass kernel for BinderEnergyGuidance (retrieval_knn), v5.

Per batch b of 16:
  d[b,n,m]   = ||binder[b,n] - target[m]||           (N=1024, M=8192)
  attract[b] = mean of the k=204 smallest per-row min-distances
  repel[b]   = sum relu(3 - d)^2
  out[b]     = 10*attract[b] + 5*repel[b]

Data-parallel over batch: 2 batches/core.  PE computes d2 into PSUM
(fp32r, K packed at partition groups 0/32/64), 4 psum tiles of
[128,2048] per chunk (128 rows x 8192 cols).  Every chunk is HYBRID so
VectorE and ScalarE both get steady work at a tunable ratio:

  U-part (XV tiles, VectorE drains): q = min(d2,9) bf16 (accum
    per-tile row-min), then u = max(q,1e-4) span pass (accum row-sum,
    4x bf16); ScalarE sqrt(u) span (accum row-sum dc).
    repel part = 9*sz + sum(u) - 6*sum(dc), exact 0 per non-clash
    element (bf16 u=9.0 -> dc=3.0 exactly).
  Q-part (4-XV tiles, ScalarE drains): dcraw = sqrt(d2+EPS) bf16;
    VectorE dcc = min(dcraw,3) (accum row-min distance); ScalarE
    Square(dcc-3) span (accum row-sum = repel part, exact 0 per
    non-clash element).

Span activations of chunk c are deferred until after chunk c+1's
ScalarE psum drain, so PE is never blocked behind them.  Staging:
zero-fills on GPSIMD memset; y/x arithmetic split VectorE/ScalarE.
Top-k attract via rank selection against a DMA-broadcast row of
min-dists; final partition sum via DMA flatten + reduce.

Self-contained: hardcodes shapes binder[16,1024,3], target[8192,3].
"""

import numpy as np
from contextlib import ExitStack

import concourse.bass as bass
import concourse.bacc as bacc
import concourse.tile as tile
from concourse import mybir
from concourse.bass_utils import run_bass_kernel_spmd

F32 = mybir.dt.float32
BF16 = mybir.dt.bfloat16
F32R = mybir.dt.float32r
AF = mybir.ActivationFunctionType
OP = mybir.AluOpType
AX = mybir.AxisListType

B, N, MT = 16, 1024, 8192
NCORES = 8
BC = B // NCORES
TOPK = 204
ATTRACT_SCALE, REPEL_SCALE = 10.0, 5.0

P = 128
NCHUNK = N // P           # 8
MTILE = 2048              # PSUM tile (4 banks)
NMT = MT // MTILE         # 4
MMF = 512
KP = 67

EPS = 0.001
PSUM_BUFS = 2
XV = 3                    # V-drained psum tiles per chunk (1..3)
UW = XV * MTILE           # U-part width
QW = MT - UW              # Q-part width

_prog_cache = {}


def build_program():
    nc = bacc.Bacc("TRN2", target_bir_lowering=False, debug=False,
                   num_devices=NCORES)
    bnd = nc.dram_tensor("bnd", [BC, 3, N], F32, kind="ExternalInput").ap()
    tgt = nc.dram_tensor("tgt", [3, MT], F32, kind="ExternalInput").ap()
    out = nc.dram_tensor("out", [BC, 1], F32, kind="ExternalOutput").ap()

    with tile.TileContext(nc) as tc, ExitStack() as ctx:
        consts = ctx.enter_context(tc.tile_pool(name="consts", bufs=1))
        work = ctx.enter_context(tc.tile_pool(name="work", bufs=1))
        tcp = ctx.enter_context(tc.tile_pool(name="tcp", bufs=2))
        psum = ctx.enter_context(tc.tile_pool(name="psum", bufs=PSUM_BUFS, space="PSUM"))
        dpool = ctx.enter_context(tc.tile_pool(name="dpool", bufs=1, space="DRAM"))

        rhs_subs = [consts.tile([KP, MTILE], F32R, name=f"rhs{k}")
                    for k in range(NMT)]
        lhsTs = [consts.tile([KP, N], F32R, name=f"lhsT_pad{b}")
                 for b in range(BC)]
        with tc.tile_pool(name="zscr", bufs=1) as zscr:
            nc.gpsimd.memset(lhsTs[0][:, :].bitcast(mybir.dt.uint32), 0)
            for k in range(NMT):
                nc.gpsimd.memset(
                    rhs_subs[k][:, :].bitcast(mybir.dt.uint32), 0)
            nc.gpsimd.memset(lhsTs[1][:, :].bitcast(mybir.dt.uint32), 0)
            def stage_lhs(b, eng):
                xs = zscr.tile([3, N], F32, name=f"xs{b}")
                nc.sync.dma_start(out=xs[:, :], in_=bnd[b, :, :])
                lhsT_pad = lhsTs[b]
                nc.gpsimd.memset(
                    lhsT_pad[64:67, :].bitcast(mybir.dt.uint32), 0x3F800000)
                if eng == 'V':
                    nc.vector.tensor_copy(lhsT_pad[0:3, :], xs[:, :])
                    nc.vector.tensor_mul(lhsT_pad[32:35, :], xs[:, :],
                                         xs[:, :])
                else:
                    nc.scalar.activation(lhsT_pad[0:3, :], xs[:, :], AF.Copy)
                    nc.scalar.activation(lhsT_pad[32:35, :], xs[:, :],
                                         AF.Square)

            def stage_rhs(k, eng):
                ksl = slice(k * MTILE, (k + 1) * MTILE)
                ysh = zscr.tile([3, MTILE], F32, name="ysh", tag="ysh")
                nc.sync.dma_start(out=ysh[:, :], in_=tgt[:, ksl])
                rs = rhs_subs[k]
                nc.gpsimd.memset(
                    rs[32:35, :].bitcast(mybir.dt.uint32), 0x3F800000)
                if eng == 'V':
                    nc.vector.tensor_scalar_mul(rs[0:3, :], ysh[:, :], -2.0)
                    nc.vector.tensor_mul(rs[64:67, :], ysh[:, :], ysh[:, :])
                else:
                    nc.scalar.activation(rs[0:3, :], ysh[:, :], AF.Copy,
                                         bias=0.0, scale=-2.0)
                    nc.scalar.activation(rs[64:67, :], ysh[:, :], AF.Square)

            stage_lhs(0, 'V')
            stage_rhs(0, 'V')
            stage_rhs(1, 'act')
            stage_rhs(2, 'act')
            stage_rhs(3, 'V')
            stage_lhs(1, 'act')

        ones128 = consts.tile([P, 1], F32)
        nc.vector.memset(ones128, 1.0)
        beps = consts.tile([P, 1], F32)
        nc.vector.memset(beps, EPS)
        bm3 = consts.tile([P, 1], F32)
        nc.vector.memset(bm3, -3.0)

        act_waste = work.tile([P, MT], BF16, name="act_waste")
        waste_r = work.tile([P, N], BF16, name="waste_r")

        pending = []

        for b in range(BC):
            lhsT = lhsTs[b]
            mdUB = work.tile([P, NCHUNK], F32, name=f"mdUB{b}")
            mdQB = work.tile([P, NCHUNK], F32, name=f"mdQB{b}")
            suB = work.tile([P, NCHUNK], F32, name=f"suB{b}")
            sdB = work.tile([P, NCHUNK], F32, name=f"sdB{b}")
            sqB = work.tile([P, NCHUNK], F32, name=f"sqB{b}")

            for c in range(NCHUNK):
                lc = lhsT[:, c * P:(c + 1) * P]
                span_u = tcp.tile([P, UW], BF16, name="span_u", tag="span_u")
                span_d = tcp.tile([P, QW], BF16, name="span_d", tag="span_d")
                us = tcp.tile([P, UW], BF16, name="us", tag="us")
                dcc = tcp.tile([P, QW], BF16, name="dcc", tag="dcc")
                mq = tcp.tile([P, XV], F32, name="mq", tag="mq")
                for k in range(NMT):
                    ps = psum.tile([P, MTILE], F32, name="ps", tag="ps")
                    for q in range(MTILE // MMF):
                        nc.tensor.matmul(
                            ps[:, q * MMF:(q + 1) * MMF], lc,
                            rhs_subs[k][:, q * MMF:(q + 1) * MMF],
                            start=True, stop=True)
                    if k < NMT - XV:
                        # act-drained Q tile
                        nc.scalar.activation(
                            span_d[:, k * MTILE:(k + 1) * MTILE], ps,
                            AF.Sqrt, bias=beps, scale=1.0)
                        for _ in range(min(2, len(pending))):
                            pending.pop(0)()
                    else:
                        kk = k - (NMT - XV)
                        nc.vector.tensor_scalar(
                            span_u[:, kk * MTILE:(kk + 1) * MTILE], ps,
                            9.0, 3.4e38, OP.min, OP.min,
                            accum_out=mq[:, kk:kk + 1])
                nc.vector.tensor_scalar(us, span_u, 1e-4, 0.0,
                                        OP.max, OP.add,
                                        accum_out=suB[:, c:c + 1])
                nc.vector.tensor_scalar(dcc, span_d, 3.0, 3.4e38,
                                        OP.min, OP.min,
                                        accum_out=mdQB[:, c:c + 1])
                nc.vector.tensor_reduce(mdUB[:, c:c + 1], mq, AX.X, OP.min)

                def mk(us_=us, dcc_=dcc, cs=c):
                    def go_sqrt():
                        nc.scalar.activation(act_waste[:, 0:UW], us_,
                                             AF.Sqrt,
                                             accum_out=sdB[:, cs:cs + 1])

                    def go_sq():
                        nc.scalar.activation(act_waste[:, UW:MT], dcc_,
                                             AF.Square, bias=bm3, scale=1.0,
                                             accum_out=sqB[:, cs:cs + 1])
                    return [go_sqrt, go_sq]
                pending.extend(mk())

            # ---- per-batch epilogue ----
            vB = work.tile([P, NCHUNK], F32, name=f"vB{b}")
            mdc = work.tile([P, NCHUNK], F32, name=f"mdc{b}")
            nc.vector.tensor_scalar(mdc, mdUB, 1e-6, None, OP.max)
            vBu = work.tile([P, NCHUNK], F32, name=f"vBu{b}")
            nc.scalar.activation(vBu, mdc, AF.Sqrt)
            nc.vector.tensor_tensor(vB, vBu, mdQB, OP.min)

            vBb = work.tile([P, NCHUNK], BF16, name=f"vBb{b}")
            nc.vector.tensor_copy(vBb, vB)
            vfl = dpool.tile([1, N], BF16, name=f"vfl{b}")
            nc.sync.dma_start(
                out=vfl[0:1, :].rearrange("p (q c) -> p q c", q=P),
                in_=vBb[:, :])
            vrep = work.tile([P, N], BF16, name=f"vrep{b}")
            vfl_bcast = bass.AP(tensor=vfl.tensor, offset=vfl.offset,
                                ap=[[0, P], vfl.ap[-1]])
            nc.sync.dma_start(out=vrep[:, :], in_=vfl_bcast)

            while pending:
                pending.pop(0)()
            rank8 = work.tile([P, NCHUNK], F32, name=f"rank8{b}")
            for c in range(NCHUNK):
                nc.vector.tensor_scalar(waste_r, vrep, vB[:, c:c + 1], 0.0,
                                        OP.is_lt, OP.add,
                                        accum_out=rank8[:, c:c + 1])
            sel8 = work.tile([P, NCHUNK], F32, name=f"sel8{b}")
            nc.vector.tensor_scalar(sel8, rank8, float(TOPK), None, OP.is_lt)
            prod8 = work.tile([P, NCHUNK], F32, name=f"prod8{b}")
            nc.vector.tensor_mul(prod8, sel8, vB)

            stack2 = work.tile([P, 2], F32, name=f"stack2{b}")
            nc.vector.tensor_reduce(stack2[:, 0:1], prod8, AX.X, OP.add)

            ru = work.tile([P, 1], F32, name=f"ru{b}")
            nc.vector.tensor_reduce(ru, suB, AX.X, OP.add)
            rd = work.tile([P, 1], F32, name=f"rd{b}")
            nc.vector.tensor_reduce(rd, sdB, AX.X, OP.add)
            rqq = work.tile([P, 1], F32, name=f"rqq{b}")
            nc.vector.tensor_reduce(rqq, sqB, AX.X, OP.add)
            t0 = work.tile([P, 1], F32, name=f"t0{b}")
            nc.vector.tensor_scalar(t0, rd, -6.0, None, OP.mult)
            t1 = work.tile([P, 1], F32, name=f"t1{b}")
            nc.vector.tensor_add(t1, t0, ru)
            t2 = work.tile([P, 1], F32, name=f"t2{b}")
            nc.vector.tensor_add(t2, t1, rqq)
            nc.vector.tensor_scalar(stack2[:, 1:2], t2,
                                    float(9.0 * UW * NCHUNK), None, OP.add)

            if b == BC - 1:
                # no matmuls follow: psum rotation can't stall
                fin = psum.tile([P, MTILE], F32, name="ps", tag="ps")
                nc.tensor.matmul(fin[0:1, 0:2], ones128, stack2,
                                 start=True, stop=True)
                esrc = fin[0:1, 0:2]
            else:
                sfl = dpool.tile([1, 2 * P], F32, name=f"sfl{b}")
                nc.sync.dma_start(
                    out=sfl[0:1, :].rearrange("p (q c) -> p q c", q=P),
                    in_=stack2[:, :])
                ssb = work.tile([1, 2 * P], F32, name=f"ssb{b}")
                nc.sync.dma_start(out=ssb, in_=sfl)
                esum = work.tile([1, 2], F32, name=f"esum{b}")
                nc.vector.tensor_reduce(
                    esum[0:1, :],
                    ssb[0:1, :].rearrange("p (q c) -> p c q", c=2),
                    AX.X, OP.add)
                esrc = esum[0:1, :]
            en = work.tile([1, 2], F32, name=f"en{b}")
            nc.vector.tensor_scalar_mul(en[0:1, 0:1], esrc[0:1, 0:1],
                                        ATTRACT_SCALE / TOPK)
            nc.vector.tensor_scalar_mul(en[0:1, 1:2], esrc[0:1, 1:2],
                                        REPEL_SCALE)
            en2 = work.tile([1, 1], F32, name=f"en2{b}")
            nc.vector.tensor_add(en2, en[0:1, 0:1], en[0:1, 1:2])
            nc.sync.dma_start(out=out[b:b + 1, 0:1], in_=en2[0:1, 0:1])

    nc.compile()
    return nc


def _get_program():
    if "nc" not in _prog_cache:
        _prog_cache["nc"] = build_program()
    return _prog_cache["nc"]


def make_in_maps(binder_trans, target_coords):
    x = np.ascontiguousarray(
        np.asarray(binder_trans, dtype=np.float32).transpose(0, 2, 1))
    y = np.ascontiguousarray(np.asarray(target_coords, dtype=np.float32).T)
    return [{"bnd": np.ascontiguousarray(x[c * BC:(c + 1) * BC]), "tgt": y}
            for c in range(NCORES)]


def kernel(binder_trans, target_coords):
    nc = _get_program()
    in_maps = make_in_maps(binder_trans, target_coords)
    res = run_bass_kernel_spmd(nc, in_maps, list(range(NCORES)))
    outs = [np.asarray(res.results[c]["out"], dtype=np.float32).reshape(BC)
            for c in range(NCORES)]
    return np.concatenate(outs).astype(np.float32)


# revision 8
# speedup vs baseline: 1.0745x; 1.0044x over previous
"""Trainium2 Bass kernel for BinderEnergyGuidance (retrieval_knn), v5.

Per batch b of 16:
  d[b,n,m]   = ||binder[b,n] - target[m]||           (N=1024, M=8192)
  attract[b] = mean of the k=204 smallest per-row min-distances
  repel[b]   = sum relu(3 - d)^2
  out[b]     = 10*attract[b] + 5*repel[b]

Data-parallel over batch: 2 batches/core.  PE computes d2 into PSUM
(fp32r, K packed at partition groups 0/32/64), 4 psum tiles of
[128,2048] per chunk (128 rows x 8192 cols).  Every chunk is HYBRID so
VectorE and ScalarE both get steady work at a tunable ratio:

  U-part (XV tiles, VectorE drains): q = min(d2,9) bf16 (accum
    per-tile row-min), then u = max(q,1e-4) span pass (accum row-sum,
    4x bf16); ScalarE sqrt(u) span (accum row-sum dc).
    repel part = 9*sz + sum(u) - 6*sum(dc), exact 0 per non-clash
    element (bf16 u=9.0 -> dc=3.0 exactly).
  Q-part (4-XV tiles, ScalarE drains): dcraw = sqrt(d2+EPS) bf16;
    VectorE dcc = min(dcraw,3) (accum row-min distance); ScalarE
    Square(dcc-3) span (accum row-sum = repel part, exact 0 per
    non-clash element).

Span activations of chunk c are deferred until after chunk c+1's
ScalarE psum drain, so PE is never blocked behind them.  Staging:
zero-fills on GPSIMD memset; y/x arithmetic split VectorE/ScalarE.
Top-k attract via rank selection against a DMA-broadcast row of
min-dists; final partition sum via DMA flatten + reduce.

Self-contained: hardcodes shapes binder[16,1024,3], target[8192,3].
"""

import numpy as np
from contextlib import ExitStack

import concourse.bass as bass
import concourse.bacc as bacc
import concourse.tile as tile
from concourse import mybir
from concourse.bass_utils import run_bass_kernel_spmd

F32 = mybir.dt.float32
BF16 = mybir.dt.bfloat16
F32R = mybir.dt.float32r
AF = mybir.ActivationFunctionType
OP = mybir.AluOpType
AX = mybir.AxisListType

B, N, MT = 16, 1024, 8192
NCORES = 8
BC = B // NCORES
TOPK = 204
ATTRACT_SCALE, REPEL_SCALE = 10.0, 5.0

P = 128
NCHUNK = N // P           # 8
MTILE = 2048              # PSUM tile (4 banks)
NMT = MT // MTILE         # 4
MMF = 512
KP = 67

EPS = 0.001
PSUM_BUFS = 2
XV = 3                    # V-drained psum tiles per chunk (1..3)
UW = XV * MTILE           # U-part width
QW = MT - UW              # Q-part width

_prog_cache = {}


def build_program():
    nc = bacc.Bacc("TRN2", target_bir_lowering=False, debug=False,
                   num_devices=NCORES)
    bnd = nc.dram_tensor("bnd", [BC, 3, N], F32, kind="ExternalInput").ap()
    tgt = nc.dram_tensor("tgt", [3, MT], F32, kind="ExternalInput").ap()
    out = nc.dram_tensor("out", [BC, 1], F32, kind="ExternalOutput").ap()

    with tile.TileContext(nc) as tc, ExitStack() as ctx:
        consts = ctx.enter_context(tc.tile_pool(name="consts", bufs=1))
        work = ctx.enter_context(tc.tile_pool(name="work", bufs=1))
        tcp = ctx.enter_context(tc.tile_pool(name="tcp", bufs=2))
        psum = ctx.enter_context(tc.tile_pool(name="psum", bufs=PSUM_BUFS, space="PSUM"))
        dpool = ctx.enter_context(tc.tile_pool(name="dpool", bufs=1, space="DRAM"))

        rhs_subs = [consts.tile([KP, MTILE], F32R, name=f"rhs{k}")
                    for k in range(NMT)]
        lhsTs = [consts.tile([KP, N], F32R, name=f"lhsT_pad{b}")
                 for b in range(BC)]
        with tc.tile_pool(name="zscr", bufs=1) as zscr:
            nc.gpsimd.memset(lhsTs[0][:, :].bitcast(mybir.dt.uint32), 0)
            for k in range(NMT):
                nc.gpsimd.memset(
                    rhs_subs[k][:, :].bitcast(mybir.dt.uint32), 0)
            nc.gpsimd.memset(lhsTs[1][:, :].bitcast(mybir.dt.uint32), 0)
            def stage_lhs(b, eng):
                xs = zscr.tile([3, N], F32, name=f"xs{b}")
                nc.sync.dma_start(out=xs[:, :], in_=bnd[b, :, :])
                lhsT_pad = lhsTs[b]
                nc.gpsimd.memset(
                    lhsT_pad[64:67, :].bitcast(mybir.dt.uint32), 0x3F800000)
                if eng == 'V':
                    nc.vector.tensor_copy(lhsT_pad[0:3, :], xs[:, :])
                    nc.vector.tensor_mul(lhsT_pad[32:35, :], xs[:, :],
                                         xs[:, :])
                else:
                    nc.scalar.activation(lhsT_pad[0:3, :], xs[:, :], AF.Copy)
                    nc.scalar.activation(lhsT_pad[32:35, :], xs[:, :],
                                         AF.Square)

            def stage_rhs(k, eng):
                ksl = slice(k * MTILE, (k + 1) * MTILE)
                ysh = zscr.tile([3, MTILE], F32, name="ysh", tag="ysh")
                nc.sync.dma_start(out=ysh[:, :], in_=tgt[:, ksl])
                rs = rhs_subs[k]
                nc.gpsimd.memset(
                    rs[32:35, :].bitcast(mybir.dt.uint32), 0x3F800000)
                if eng == 'V':
                    nc.vector.tensor_scalar_mul(rs[0:3, :], ysh[:, :], -2.0)
                    nc.vector.tensor_mul(rs[64:67, :], ysh[:, :], ysh[:, :])
                else:
                    nc.scalar.activation(rs[0:3, :], ysh[:, :], AF.Copy,
                                         bias=0.0, scale=-2.0)
                    nc.scalar.activation(rs[64:67, :], ysh[:, :], AF.Square)

            stage_lhs(0, 'V')
            stage_rhs(0, 'V')
            stage_rhs(1, 'act')
            stage_rhs(2, 'act')
            stage_rhs(3, 'act')
            stage_lhs(1, 'V')

        ones128 = consts.tile([P, 1], F32)
        nc.vector.memset(ones128, 1.0)
        beps = consts.tile([P, 1], F32)
        nc.vector.memset(beps, EPS)
        bm3 = consts.tile([P, 1], F32)
        nc.vector.memset(bm3, -3.0)

        act_waste = work.tile([P, MT], BF16, name="act_waste")
        waste_r = work.tile([P, N], BF16, name="waste_r")

        pending = []

        for b in range(BC):
            lhsT = lhsTs[b]
            mdUB = work.tile([P, NCHUNK], F32, name=f"mdUB{b}")
            mdQB = work.tile([P, NCHUNK], F32, name=f"mdQB{b}")
            suB = work.tile([P, NCHUNK], F32, name=f"suB{b}")
            sdB = work.tile([P, NCHUNK], F32, name=f"sdB{b}")
            sqB = work.tile([P, NCHUNK], F32, name=f"sqB{b}")

            for c in range(NCHUNK):
                lc = lhsT[:, c * P:(c + 1) * P]
                span_u = tcp.tile([P, UW], BF16, name="span_u", tag="span_u")
                span_d = tcp.tile([P, QW], BF16, name="span_d", tag="span_d")
                us = tcp.tile([P, UW], BF16, name="us", tag="us")
                dcc = tcp.tile([P, QW], BF16, name="dcc", tag="dcc")
                mq = tcp.tile([P, XV], F32, name="mq", tag="mq")
                for k in range(NMT):
                    ps = psum.tile([P, MTILE], F32, name="ps", tag="ps")
                    for q in range(MTILE // MMF):
                        nc.tensor.matmul(
                            ps[:, q * MMF:(q + 1) * MMF], lc,
                            rhs_subs[k][:, q * MMF:(q + 1) * MMF],
                            start=True, stop=True)
                    if k < NMT - XV:
                        # act-drained Q tile
                        nc.scalar.activation(
                            span_d[:, k * MTILE:(k + 1) * MTILE], ps,
                            AF.Sqrt, bias=beps, scale=1.0)
                        for _ in range(min(2, len(pending))):
                            pending.pop(0)()
                    else:
                        kk = k - (NMT - XV)
                        nc.vector.tensor_scalar(
                            span_u[:, kk * MTILE:(kk + 1) * MTILE], ps,
                            9.0, 3.4e38, OP.min, OP.min,
                            accum_out=mq[:, kk:kk + 1])
                nc.vector.tensor_scalar(us, span_u, 1e-4, 0.0,
                                        OP.max, OP.add,
                                        accum_out=suB[:, c:c + 1])
                nc.vector.tensor_scalar(dcc, span_d, 3.0, 3.4e38,
                                        OP.min, OP.min,
                                        accum_out=mdQB[:, c:c + 1])
                nc.vector.tensor_reduce(mdUB[:, c:c + 1], mq, AX.X, OP.min)

                def mk(us_=us, dcc_=dcc, cs=c):
                    def go_sqrt():
                        nc.scalar.activation(act_waste[:, 0:UW], us_,
                                             AF.Sqrt,
                                             accum_out=sdB[:, cs:cs + 1])

                    def go_sq():
                        nc.scalar.activation(act_waste[:, UW:MT], dcc_,
                                             AF.Square, bias=bm3, scale=1.0,
                                             accum_out=sqB[:, cs:cs + 1])
                    return [go_sqrt, go_sq]
                pending.extend(mk())

            # ---- per-batch epilogue ----
            vB = work.tile([P, NCHUNK], F32, name=f"vB{b}")
            mdc = work.tile([P, NCHUNK], F32, name=f"mdc{b}")
            nc.vector.tensor_scalar(mdc, mdUB, 1e-6, None, OP.max)
            vBu = work.tile([P, NCHUNK], F32, name=f"vBu{b}")
            nc.scalar.activation(vBu, mdc, AF.Sqrt)
            nc.vector.tensor_tensor(vB, vBu, mdQB, OP.min)

            vBb = work.tile([P, NCHUNK], BF16, name=f"vBb{b}")
            nc.vector.tensor_copy(vBb, vB)
            vfl = dpool.tile([1, N], BF16, name=f"vfl{b}")
            nc.sync.dma_start(
                out=vfl[0:1, :].rearrange("p (q c) -> p q c", q=P),
                in_=vBb[:, :])
            vrep = work.tile([P, N], BF16, name=f"vrep{b}")
            vfl_bcast = bass.AP(tensor=vfl.tensor, offset=vfl.offset,
                                ap=[[0, P], vfl.ap[-1]])
            nc.sync.dma_start(out=vrep[:, :], in_=vfl_bcast)

            while pending:
                pending.pop(0)()
            rank8 = work.tile([P, NCHUNK], F32, name=f"rank8{b}")
            for c in range(NCHUNK):
                nc.vector.tensor_scalar(waste_r, vrep, vB[:, c:c + 1], 0.0,
                                        OP.is_lt, OP.add,
                                        accum_out=rank8[:, c:c + 1])
            sel8 = work.tile([P, NCHUNK], F32, name=f"sel8{b}")
            nc.vector.tensor_scalar(sel8, rank8, float(TOPK), None, OP.is_lt)
            prod8 = work.tile([P, NCHUNK], F32, name=f"prod8{b}")
            nc.vector.tensor_mul(prod8, sel8, vB)

            stack2 = work.tile([P, 2], F32, name=f"stack2{b}")
            nc.vector.tensor_reduce(stack2[:, 0:1], prod8, AX.X, OP.add)

            ru = work.tile([P, 1], F32, name=f"ru{b}")
            nc.vector.tensor_reduce(ru, suB, AX.X, OP.add)
            rd = work.tile([P, 1], F32, name=f"rd{b}")
            nc.vector.tensor_reduce(rd, sdB, AX.X, OP.add)
            rqq = work.tile([P, 1], F32, name=f"rqq{b}")
            nc.vector.tensor_reduce(rqq, sqB, AX.X, OP.add)
            t0 = work.tile([P, 1], F32, name=f"t0{b}")
            nc.vector.tensor_scalar(t0, rd, -6.0, None, OP.mult)
            t1 = work.tile([P, 1], F32, name=f"t1{b}")
            nc.vector.tensor_add(t1, t0, ru)
            t2 = work.tile([P, 1], F32, name=f"t2{b}")
            nc.vector.tensor_add(t2, t1, rqq)
            nc.vector.tensor_scalar(stack2[:, 1:2], t2,
                                    float(9.0 * UW * NCHUNK), None, OP.add)

            if b == BC - 1:
                # no matmuls follow: psum rotation can't stall
                fin = psum.tile([P, MTILE], F32, name="ps", tag="ps")
                nc.tensor.matmul(fin[0:1, 0:2], ones128, stack2,
                                 start=True, stop=True)
                esrc = fin[0:1, 0:2]
            else:
                sfl = dpool.tile([1, 2 * P], F32, name=f"sfl{b}")
                nc.sync.dma_start(
                    out=sfl[0:1, :].rearrange("p (q c) -> p q c", q=P),
                    in_=stack2[:, :])
                ssb = work.tile([1, 2 * P], F32, name=f"ssb{b}")
                nc.sync.dma_start(out=ssb, in_=sfl)
                esum = work.tile([1, 2], F32, name=f"esum{b}")
                nc.vector.tensor_reduce(
                    esum[0:1, :],
                    ssb[0:1, :].rearrange("p (q c) -> p c q", c=2),
                    AX.X, OP.add)
                esrc = esum[0:1, :]
            en = work.tile([1, 2], F32, name=f"en{b}")
            nc.vector.tensor_scalar_mul(en[0:1, 0:1], esrc[0:1, 0:1],
                                        ATTRACT_SCALE / TOPK)
            nc.vector.tensor_scalar_mul(en[0:1, 1:2], esrc[0:1, 1:2],
                                        REPEL_SCALE)
            en2 = work.tile([1, 1], F32, name=f"en2{b}")
            nc.vector.tensor_add(en2, en[0:1, 0:1], en[0:1, 1:2])
            nc.sync.dma_start(out=out[b:b + 1, 0:1], in_=en2[0:1, 0:1])

    nc.compile()
    return nc


def _get_program():
    if "nc" not in _prog_cache:
        _prog_cache["nc"] = build_program()
    return _prog_cache["nc"]


def make_in_maps(binder_trans, target_coords):
    x = np.ascontiguousarray(
        np.asarray(binder_trans, dtype=np.float32).transpose(0, 2, 1))
    y = np.ascontiguousarray(np.asarray(target_coords, dtype=np.float32).T)
    return [{"bnd": np.ascontiguousarray(x[c * BC:(c + 1) * BC]), "tgt": y}
            for c in range(NCORES)]


def kernel(binder_trans, target_coords):
    nc = _get_program()
    in_maps = make_in_maps(binder_trans, target_coords)
    res = run_bass_kernel_spmd(nc, in_maps, list(range(NCORES)))
    outs = [np.asarray(res.results[c]["out"], dtype=np.float32).reshape(BC)
            for c in range(NCORES)]
    return np.concatenate(outs).astype(np.float32)


# revision 9
# speedup vs baseline: 1.1065x; 1.0297x over previous
"""Trainium2 Bass kernel for BinderEnergyGuidance (retrieval_knn), v5.

Per batch b of 16:
  d[b,n,m]   = ||binder[b,n] - target[m]||           (N=1024, M=8192)
  attract[b] = mean of the k=204 smallest per-row min-distances
  repel[b]   = sum relu(3 - d)^2
  out[b]     = 10*attract[b] + 5*repel[b]

Data-parallel over batch: 2 batches/core.  PE computes d2 into PSUM
(fp32r, K packed at partition groups 0/32/64), 4 psum tiles of
[128,2048] per chunk (128 rows x 8192 cols).  Every chunk is HYBRID so
VectorE and ScalarE both get steady work at a tunable ratio:

  U-part (XV tiles, VectorE drains): q = min(d2,9) bf16 (accum
    per-tile row-min), then u = max(q,1e-4) span pass (accum row-sum,
    4x bf16); ScalarE sqrt(u) span (accum row-sum dc).
    repel part = 9*sz + sum(u) - 6*sum(dc), exact 0 per non-clash
    element (bf16 u=9.0 -> dc=3.0 exactly).
  Q-part (4-XV tiles, ScalarE drains): dcraw = sqrt(d2+EPS) bf16;
    VectorE dcc = min(dcraw,3) (accum row-min distance); ScalarE
    Square(dcc-3) span (accum row-sum = repel part, exact 0 per
    non-clash element).

Span activations of chunk c are deferred until after chunk c+1's
ScalarE psum drain, so PE is never blocked behind them.  Staging:
zero-fills on GPSIMD memset; y/x arithmetic split VectorE/ScalarE.
Top-k attract via rank selection against a DMA-broadcast row of
min-dists; final partition sum via DMA flatten + reduce.

Self-contained: hardcodes shapes binder[16,1024,3], target[8192,3].
"""

import numpy as np
from contextlib import ExitStack

import concourse.bass as bass
import concourse.bacc as bacc
import concourse.tile as tile
from concourse import mybir
from concourse.bass_utils import run_bass_kernel_spmd

F32 = mybir.dt.float32
BF16 = mybir.dt.bfloat16
F32R = mybir.dt.float32r
AF = mybir.ActivationFunctionType
OP = mybir.AluOpType
AX = mybir.AxisListType

B, N, MT = 16, 1024, 8192
NCORES = 8
BC = B // NCORES
TOPK = 204
ATTRACT_SCALE, REPEL_SCALE = 10.0, 5.0

P = 128
NCHUNK = N // P           # 8
MTILE = 2048              # PSUM tile (4 banks)
NMT = MT // MTILE         # 4
MMF = 512
KP = 67

EPS = 0.001
PSUM_BUFS = 2
XV = 3                    # V-drained psum tiles per chunk (1..3)
UW = XV * MTILE           # U-part width
QW = MT - UW              # Q-part width

_prog_cache = {}


def build_program():
    nc = bacc.Bacc("TRN2", target_bir_lowering=False, debug=False,
                   num_devices=NCORES)
    bnd = nc.dram_tensor("bnd", [BC, 3, N], F32, kind="ExternalInput").ap()
    tgt = nc.dram_tensor("tgt", [3, MT], F32, kind="ExternalInput").ap()
    out = nc.dram_tensor("out", [BC, 1], F32, kind="ExternalOutput").ap()

    with tile.TileContext(nc) as tc, ExitStack() as ctx:
        consts = ctx.enter_context(tc.tile_pool(name="consts", bufs=1))
        work = ctx.enter_context(tc.tile_pool(name="work", bufs=1))
        tcp = ctx.enter_context(tc.tile_pool(name="tcp", bufs=2))
        psum = ctx.enter_context(tc.tile_pool(name="psum", bufs=PSUM_BUFS, space="PSUM"))
        dpool = ctx.enter_context(tc.tile_pool(name="dpool", bufs=1, space="DRAM"))

        rhs_subs = [consts.tile([KP, MTILE], F32R, name=f"rhs{k}")
                    for k in range(NMT)]
        lhsTs = [consts.tile([KP, N], F32R, name=f"lhsT_pad{b}")
                 for b in range(BC)]
        with tc.tile_pool(name="zscr", bufs=1) as zscr:
            nc.gpsimd.memset(lhsTs[0][:, :].bitcast(mybir.dt.uint32), 0)
            for k in range(NMT):
                nc.gpsimd.memset(
                    rhs_subs[k][:, :].bitcast(mybir.dt.uint32), 0)
            nc.gpsimd.memset(lhsTs[1][:, :].bitcast(mybir.dt.uint32), 0)
            def stage_lhs(b, eng):
                xs = zscr.tile([3, N], F32, name=f"xs{b}")
                nc.sync.dma_start(out=xs[:, :], in_=bnd[b, :, :])
                lhsT_pad = lhsTs[b]
                nc.gpsimd.memset(
                    lhsT_pad[64:67, :].bitcast(mybir.dt.uint32), 0x3F800000)
                if eng == 'V':
                    nc.vector.tensor_copy(lhsT_pad[0:3, :], xs[:, :])
                    nc.vector.tensor_mul(lhsT_pad[32:35, :], xs[:, :],
                                         xs[:, :])
                else:
                    nc.scalar.activation(lhsT_pad[0:3, :], xs[:, :], AF.Copy)
                    nc.scalar.activation(lhsT_pad[32:35, :], xs[:, :],
                                         AF.Square)

            def stage_rhs(k, eng):
                ksl = slice(k * MTILE, (k + 1) * MTILE)
                ysh = zscr.tile([3, MTILE], F32, name="ysh", tag="ysh")
                nc.sync.dma_start(out=ysh[:, :], in_=tgt[:, ksl])
                rs = rhs_subs[k]
                nc.gpsimd.memset(
                    rs[32:35, :].bitcast(mybir.dt.uint32), 0x3F800000)
                if eng == 'V':
                    nc.vector.tensor_scalar_mul(rs[0:3, :], ysh[:, :], -2.0)
                    nc.vector.tensor_mul(rs[64:67, :], ysh[:, :], ysh[:, :])
                else:
                    nc.scalar.activation(rs[0:3, :], ysh[:, :], AF.Copy,
                                         bias=0.0, scale=-2.0)
                    nc.scalar.activation(rs[64:67, :], ysh[:, :], AF.Square)

            stage_rhs(0, 'V')
            stage_lhs(0, 'V')
            stage_rhs(1, 'act')
            stage_rhs(2, 'act')
            stage_rhs(3, 'act')
            stage_lhs(1, 'V')

        ones128 = consts.tile([P, 1], F32)
        nc.vector.memset(ones128, 1.0)
        beps = consts.tile([P, 1], F32)
        nc.vector.memset(beps, EPS)
        bm3 = consts.tile([P, 1], F32)
        nc.vector.memset(bm3, -3.0)

        act_waste = work.tile([P, MT], BF16, name="act_waste")
        waste_r = work.tile([P, N], BF16, name="waste_r")

        pending = []

        for b in range(BC):
            lhsT = lhsTs[b]
            mdUB = work.tile([P, NCHUNK], F32, name=f"mdUB{b}")
            mdQB = work.tile([P, NCHUNK], F32, name=f"mdQB{b}")
            suB = work.tile([P, NCHUNK], F32, name=f"suB{b}")
            sdB = work.tile([P, NCHUNK], F32, name=f"sdB{b}")
            sqB = work.tile([P, NCHUNK], F32, name=f"sqB{b}")

            for c in range(NCHUNK):
                lc = lhsT[:, c * P:(c + 1) * P]
                span_u = tcp.tile([P, UW], BF16, name="span_u", tag="span_u")
                span_d = tcp.tile([P, QW], BF16, name="span_d", tag="span_d")
                us = tcp.tile([P, UW], BF16, name="us", tag="us")
                dcc = tcp.tile([P, QW], BF16, name="dcc", tag="dcc")
                mq = tcp.tile([P, XV], F32, name="mq", tag="mq")
                for k in range(NMT):
                    ps = psum.tile([P, MTILE], F32, name="ps", tag="ps")
                    for q in range(MTILE // MMF):
                        nc.tensor.matmul(
                            ps[:, q * MMF:(q + 1) * MMF], lc,
                            rhs_subs[k][:, q * MMF:(q + 1) * MMF],
                            start=True, stop=True)
                    if k < NMT - XV:
                        # act-drained Q tile
                        nc.scalar.activation(
                            span_d[:, k * MTILE:(k + 1) * MTILE], ps,
                            AF.Sqrt, bias=beps, scale=1.0)
                        for _ in range(min(2, len(pending))):
                            pending.pop(0)()
                    else:
                        kk = k - (NMT - XV)
                        nc.vector.tensor_scalar(
                            span_u[:, kk * MTILE:(kk + 1) * MTILE], ps,
                            9.0, 3.4e38, OP.min, OP.min,
                            accum_out=mq[:, kk:kk + 1])
                nc.vector.tensor_scalar(us, span_u, 1e-4, 0.0,
                                        OP.max, OP.add,
                                        accum_out=suB[:, c:c + 1])
                nc.vector.tensor_scalar(dcc, span_d, 3.0, 3.4e38,
                                        OP.min, OP.min,
                                        accum_out=mdQB[:, c:c + 1])
                nc.vector.tensor_reduce(mdUB[:, c:c + 1], mq, AX.X, OP.min)

                def mk(us_=us, dcc_=dcc, cs=c):
                    def go_sqrt():
                        nc.scalar.activation(act_waste[:, 0:UW], us_,
                                             AF.Sqrt,
                                             accum_out=sdB[:, cs:cs + 1])

                    def go_sq():
                        nc.scalar.activation(act_waste[:, UW:MT], dcc_,
                                             AF.Square, bias=bm3, scale=1.0,
                                             accum_out=sqB[:, cs:cs + 1])
                    return [go_sqrt, go_sq]
                pending.extend(mk())

            # ---- per-batch epilogue ----
            vB = work.tile([P, NCHUNK], F32, name=f"vB{b}")
            mdc = work.tile([P, NCHUNK], F32, name=f"mdc{b}")
            nc.vector.tensor_scalar(mdc, mdUB, 1e-6, None, OP.max)
            vBu = work.tile([P, NCHUNK], F32, name=f"vBu{b}")
            nc.scalar.activation(vBu, mdc, AF.Sqrt)
            nc.vector.tensor_tensor(vB, vBu, mdQB, OP.min)

            vBb = work.tile([P, NCHUNK], BF16, name=f"vBb{b}")
            nc.vector.tensor_copy(vBb, vB)
            vfl = dpool.tile([1, N], BF16, name=f"vfl{b}")
            nc.sync.dma_start(
                out=vfl[0:1, :].rearrange("p (q c) -> p q c", q=P),
                in_=vBb[:, :])
            vrep = work.tile([P, N], BF16, name=f"vrep{b}")
            vfl_bcast = bass.AP(tensor=vfl.tensor, offset=vfl.offset,
                                ap=[[0, P], vfl.ap[-1]])
            nc.sync.dma_start(out=vrep[:, :], in_=vfl_bcast)

            while pending:
                pending.pop(0)()
            rank8 = work.tile([P, NCHUNK], F32, name=f"rank8{b}")
            for c in range(NCHUNK):
                nc.vector.tensor_scalar(waste_r, vrep, vB[:, c:c + 1], 0.0,
                                        OP.is_lt, OP.add,
                                        accum_out=rank8[:, c:c + 1])
            sel8 = work.tile([P, NCHUNK], F32, name=f"sel8{b}")
            nc.vector.tensor_scalar(sel8, rank8, float(TOPK), None, OP.is_lt)
            prod8 = work.tile([P, NCHUNK], F32, name=f"prod8{b}")
            nc.vector.tensor_mul(prod8, sel8, vB)

            stack2 = work.tile([P, 2], F32, name=f"stack2{b}")
            nc.vector.tensor_reduce(stack2[:, 0:1], prod8, AX.X, OP.add)

            ru = work.tile([P, 1], F32, name=f"ru{b}")
            nc.vector.tensor_reduce(ru, suB, AX.X, OP.add)
            rd = work.tile([P, 1], F32, name=f"rd{b}")
            nc.vector.tensor_reduce(rd, sdB, AX.X, OP.add)
            rqq = work.tile([P, 1], F32, name=f"rqq{b}")
            nc.vector.tensor_reduce(rqq, sqB, AX.X, OP.add)
            t0 = work.tile([P, 1], F32, name=f"t0{b}")
            nc.vector.tensor_scalar(t0, rd, -6.0, None, OP.mult)
            t1 = work.tile([P, 1], F32, name=f"t1{b}")
            nc.vector.tensor_add(t1, t0, ru)
            t2 = work.tile([P, 1], F32, name=f"t2{b}")
            nc.vector.tensor_add(t2, t1, rqq)
            nc.vector.tensor_scalar(stack2[:, 1:2], t2,
                                    float(9.0 * UW * NCHUNK), None, OP.add)

            if b == BC - 1:
                # no matmuls follow: psum rotation can't stall
                fin = psum.tile([P, MTILE], F32, name="ps", tag="ps")
                nc.tensor.matmul(fin[0:1, 0:2], ones128, stack2,
                                 start=True, stop=True)
                esrc = fin[0:1, 0:2]
            else:
                sfl = dpool.tile([1, 2 * P], F32, name=f"sfl{b}")
                nc.sync.dma_start(
                    out=sfl[0:1, :].rearrange("p (q c) -> p q c", q=P),
                    in_=stack2[:, :])
                ssb = work.tile([1, 2 * P], F32, name=f"ssb{b}")
                nc.sync.dma_start(out=ssb, in_=sfl)
                esum = work.tile([1, 2], F32, name=f"esum{b}")
                nc.vector.tensor_reduce(
                    esum[0:1, :],
                    ssb[0:1, :].rearrange("p (q c) -> p c q", c=2),
                    AX.X, OP.add)
                esrc = esum[0:1, :]
            en = work.tile([1, 2], F32, name=f"en{b}")
            nc.vector.tensor_scalar_mul(en[0:1, 0:1], esrc[0:1, 0:1],
                                        ATTRACT_SCALE / TOPK)
            nc.vector.tensor_scalar_mul(en[0:1, 1:2], esrc[0:1, 1:2],
                                        REPEL_SCALE)
            en2 = work.tile([1, 1], F32, name=f"en2{b}")
            nc.vector.tensor_add(en2, en[0:1, 0:1], en[0:1, 1:2])
            nc.sync.dma_start(out=out[b:b + 1, 0:1], in_=en2[0:1, 0:1])

    nc.compile()
    return nc


def _get_program():
    if "nc" not in _prog_cache:
        _prog_cache["nc"] = build_program()
    return _prog_cache["nc"]


def make_in_maps(binder_trans, target_coords):
    x = np.ascontiguousarray(
        np.asarray(binder_trans, dtype=np.float32).transpose(0, 2, 1))
    y = np.ascontiguousarray(np.asarray(target_coords, dtype=np.float32).T)
    return [{"bnd": np.ascontiguousarray(x[c * BC:(c + 1) * BC]), "tgt": y}
            for c in range(NCORES)]


def kernel(binder_trans, target_coords):
    nc = _get_program()
    in_maps = make_in_maps(binder_trans, target_coords)
    res = run_bass_kernel_spmd(nc, in_maps, list(range(NCORES)))
    outs = [np.asarray(res.results[c]["out"], dtype=np.float32).reshape(BC)
            for c in range(NCORES)]
    return np.concatenate(outs).astype(np.float32)
